# revision 1
# baseline (speedup 1.0000x reference)
"""TRN2 Bass/Tile kernel for nn_Loss_58317065945194.

Loss: per-sample EMD with r=2 over C=10 channels:
    d = p - q                       # [B, C]
    S = cumsum(d, axis=1)           # per-sample prefix sums
    per_sample = sqrt(mean(S**2))   # [B]
    out = mean(per_sample)          # scalar

Strategy (pure data parallel, 8 cores):
  - Shard B across 8 cores; per core reshape the [Bs, 10] shard to
    [128 partitions, 20480] (each partition holds 2048 whole samples,
    10 contiguous values each). Inputs are cast to fp16 host-side
    (halves HBM traffic; scan state stays fp32 internally).
  - Per chunk of W samples/partition:
      * Vector:  one tensor_tensor_scan fuses the subtract with the
                 running prefix sum: S[t] = (p[t] + state) - q[t]
                 (state kept fp32 internally; output fp16)
      * GpSimd/Vector (alternating): per-sample prefix sums recovered
                 by subtracting each sample's start boundary
                 (broadcast AP with step 0)
      * Scalar:  sq = c^2  (in place)
      * Vector:  U[g] = sum_j sq[g, j]   (3D AP, reduce axis=X)
      * Scalar:  loss = sqrt(U / C), accum_out -> per-chunk column
  - Each core returns [128, NCHUNK] fp32 partial sums of per-sample
    losses; the host sums all partials and divides by B (replaces the
    all-reduce).
"""

import sys

import numpy as np

if "/opt/trn_rl_repo" not in sys.path:
    sys.path.insert(0, "/opt/trn_rl_repo")

N_CORES = 8
B, C = 2097152, 10
BS = B // N_CORES        # samples per core shard
P = 128                  # SBUF partitions
FPP = BS * C // P        # elems per partition (20480)
W = 256                  # samples per chunk per partition
CW = W * C               # chunk free width (2560)
NCHUNK = FPP // CW       # chunks per core (8)

_cache = {}


def _build_program():
    import concourse.tile as tile
    from concourse import bacc, mybir

    f32, f16 = mybir.dt.float32, mybir.dt.float16
    Alu = mybir.AluOpType
    Act = mybir.ActivationFunctionType

    nc = bacc.Bacc(
        "TRN2", target_bir_lowering=False, debug=False, num_devices=N_CORES
    )
    p_d = nc.dram_tensor("p", [P, FPP], f16, kind="ExternalInput").ap()
    q_d = nc.dram_tensor("q", [P, FPP], f16, kind="ExternalInput").ap()
    o_d = nc.dram_tensor("partial", [P, NCHUNK], f32, kind="ExternalOutput").ap()

    with tile.TileContext(nc) as tc:
        with (
            tc.tile_pool(name="io", bufs=4) as io,
            tc.tile_pool(name="work", bufs=4) as work,
            tc.tile_pool(name="small", bufs=2) as small,
            tc.tile_pool(name="accp", bufs=1) as accp,
        ):
            acc = accp.tile([P, NCHUNK], f32)
            for ci in range(NCHUNK):
                pt = io.tile([P, CW], f16, tag="p")
                qt = io.tile([P, CW], f16, tag="q")
                nc.sync.dma_start(pt[:], p_d[:, ci * CW : (ci + 1) * CW])
                nc.sync.dma_start(qt[:], q_d[:, ci * CW : (ci + 1) * CW])

                # fused subtract + running prefix sum on Vector:
                # S[8+t] = (p[t] + state) - q[t]; S[7] = 0 (memset).
                # Scan output starts at offset 8 (16B) to keep it aligned.
                # S crosses sample boundaries; fixed up below.
                S = work.tile([P, CW + 8], f16, tag="S")
                nc.gpsimd.memset(S[:, 7:8], 0.0)
                nc.vector.tensor_tensor_scan(
                    S[:, 8:], pt[:], qt[:], 0.0, Alu.add, Alu.subtract
                )

                # per-sample prefix sums: c[g, j] = S[8+10g+j] - S[8+10g-1]
                # (broadcast subtract; alternate gpsimd/vector to balance)
                s3 = S[:, 8:].rearrange("p (w c) -> p w c", c=C)
                b3 = S[:, 7 : 7 + CW : C].unsqueeze(2).broadcast_to((P, W, C))
                cs = work.tile([P, CW], f16, tag="cs")
                cs3 = cs[:].rearrange("p (w c) -> p w c", c=C)
                eng = nc.gpsimd if ci % 2 == 0 else nc.vector
                eng.tensor_tensor(cs3, s3, b3, Alu.subtract)

                # square in place on Scalar engine
                nc.scalar.activation(cs[:], cs[:], Act.Square)

                # U[g] = sum_j c[g, j]^2
                cs3 = cs[:].rearrange("p (w c) -> p w c", c=C)
                U = small.tile([P, W], f32, tag="U")
                nc.vector.tensor_reduce(
                    U[:], cs3, axis=mybir.AxisListType.X, op=Alu.add
                )

                # loss[g] = sqrt(U[g] / C); acc[:, ci] = sum_g loss[g]
                lt = small.tile([P, W], f32, tag="loss")
                nc.scalar.activation(
                    lt[:], U[:], Act.Sqrt, scale=1.0 / C,
                    accum_out=acc[:, ci : ci + 1],
                )
            nc.sync.dma_start(o_d[:], acc[:])
    nc.compile()
    return nc


def _make_in_maps(p, q):
    p = np.asarray(p, dtype=np.float32).reshape(B, C).astype(np.float16)
    q = np.asarray(q, dtype=np.float32).reshape(B, C).astype(np.float16)
    in_maps = []
    for i in range(N_CORES):
        in_maps.append(
            {
                "p": np.ascontiguousarray(p[i * BS : (i + 1) * BS]).reshape(P, FPP),
                "q": np.ascontiguousarray(q[i * BS : (i + 1) * BS]).reshape(P, FPP),
            }
        )
    return in_maps


def kernel(p, q, r):
    assert int(r) == 2, f"kernel specialized for r=2, got {r}"
    if "nc" not in _cache:
        _cache["nc"] = _build_program()
    nc = _cache["nc"]

    in_maps = _make_in_maps(p, q)

    from concourse.bass_utils import run_bass_kernel_spmd

    res = run_bass_kernel_spmd(nc, in_maps, list(range(N_CORES)))
    total = 0.0
    for r_ in res.results:
        total += r_["partial"].astype(np.float64).sum()
    return np.float32(total / B)



# revision 2
# speedup vs baseline: 1.1297x; 1.1297x over previous
"""TRN2 Bass/Tile kernel for nn_Loss_58317065945194.

Loss: per-sample EMD with r=2 over C=10 channels:
    d = p - q                       # [B, C]
    S = cumsum(d, axis=1)           # per-sample prefix sums
    per_sample = sqrt(mean(S**2))   # [B]
    out = mean(per_sample)          # scalar

Strategy (pure data parallel, 8 cores):
  - Shard B across 8 cores. Host casts to fp16 and lays each core's
    shard out channel-major per chunk: [128 partitions][NCHUNK][C=10][W]
    so every on-device op is a 2D unit-stride access pattern.
  - Per chunk:
      * Vector (stt, 4x mode): S = (p bypass 0) - q, fused over the
        whole chunk; then 9 chained adds S_c += S_{c-1} compute the
        per-sample prefix sums with no scan and no boundary fixup.
      * Scalar ACT: sq_c = S_c^2 per channel (fp16)
      * Vector (stt, 4x): U += sq_c (fp16 accumulator)
      * Scalar ACT: loss = sqrt(U / C), accum_out -> per-chunk partial
  - Each core returns [128, NCHUNK] fp32 partial sums of per-sample
    losses; the host sums all partials and divides by B.
"""

import sys

import numpy as np

if "/opt/trn_rl_repo" not in sys.path:
    sys.path.insert(0, "/opt/trn_rl_repo")

N_CORES = 8
B, C = 2097152, 10
BS = B // N_CORES        # samples per core shard (262144)
P = 128                  # SBUF partitions
SPP = BS // P            # samples per partition (2048)
W = 1024                 # samples per chunk per partition
NCHUNK = SPP // W        # chunks per core (2)
CW = W * C               # chunk free width (10240)
FPP = SPP * C            # elems per partition (20480)

_cache = {}


def _build_program():
    import concourse.tile as tile
    from concourse import bacc, mybir

    f32, f16 = mybir.dt.float32, mybir.dt.float16
    Alu = mybir.AluOpType
    Act = mybir.ActivationFunctionType

    nc = bacc.Bacc(
        "TRN2", target_bir_lowering=False, debug=False, num_devices=N_CORES
    )
    p_d = nc.dram_tensor("p", [P, FPP], f16, kind="ExternalInput").ap()
    q_d = nc.dram_tensor("q", [P, FPP], f16, kind="ExternalInput").ap()
    o_d = nc.dram_tensor("partial", [P, NCHUNK], f32, kind="ExternalOutput").ap()

    with tile.TileContext(nc) as tc:
        with (
            tc.tile_pool(name="io", bufs=2) as io,
            tc.tile_pool(name="work", bufs=2) as work,
            tc.tile_pool(name="small", bufs=2) as small,
            tc.tile_pool(name="accp", bufs=1) as accp,
        ):
            acc = accp.tile([P, NCHUNK], f32)
            for ci in range(NCHUNK):
                pt = io.tile([P, CW], f16, tag="p")
                qt = io.tile([P, CW], f16, tag="q")
                nc.sync.dma_start(pt[:], p_d[:, ci * CW : (ci + 1) * CW])
                nc.sync.dma_start(qt[:], q_d[:, ci * CW : (ci + 1) * CW])

                # S = (p bypass 0.0) - q over the whole chunk (stt, 4x)
                S = work.tile([P, CW], f16, tag="S")
                nc.vector.scalar_tensor_tensor(
                    S[:], pt[:], 0.0, qt[:], Alu.bypass, Alu.subtract
                )

                # chained prefix adds along c: S_c += S_{c-1}
                for c in range(1, C):
                    nc.vector.scalar_tensor_tensor(
                        S[:, c * W : (c + 1) * W],
                        S[:, c * W : (c + 1) * W],
                        0.0,
                        S[:, (c - 1) * W : c * W],
                        Alu.bypass,
                        Alu.add,
                    )

                # squares on ACT; U accumulated on Vector (stt, 4x)
                SQ = work.tile([P, CW], f16, tag="SQ")
                U = small.tile([P, W], f16, tag="U")
                for c in range(C):
                    nc.scalar.activation(
                        SQ[:, c * W : (c + 1) * W],
                        S[:, c * W : (c + 1) * W],
                        Act.Square,
                    )
                    if c == 0:
                        pass
                    elif c == 1:
                        nc.vector.scalar_tensor_tensor(
                            U[:], SQ[:, 0:W], 0.0, SQ[:, W : 2 * W],
                            Alu.bypass, Alu.add,
                        )
                    else:
                        nc.vector.scalar_tensor_tensor(
                            U[:], U[:], 0.0, SQ[:, c * W : (c + 1) * W],
                            Alu.bypass, Alu.add,
                        )

                # loss[g] = sqrt(U[g] / C); acc[:, ci] = sum_g loss[g]
                lt = small.tile([P, W], f32, tag="loss")
                nc.scalar.activation(
                    lt[:], U[:], Act.Sqrt, scale=1.0 / C,
                    accum_out=acc[:, ci : ci + 1],
                )
            nc.sync.dma_start(o_d[:], acc[:])
    nc.compile()
    return nc


def _make_in_maps(p, q):
    p = np.asarray(p, dtype=np.float32).reshape(B, C).astype(np.float16)
    q = np.asarray(q, dtype=np.float32).reshape(B, C).astype(np.float16)

    def prep(a, i):
        sh = a[i * BS : (i + 1) * BS].reshape(P, NCHUNK, W, C)
        return np.ascontiguousarray(sh.transpose(0, 1, 3, 2)).reshape(P, FPP)

    in_maps = []
    for i in range(N_CORES):
        in_maps.append({"p": prep(p, i), "q": prep(q, i)})
    return in_maps


def kernel(p, q, r):
    assert int(r) == 2, f"kernel specialized for r=2, got {r}"
    if "nc" not in _cache:
        _cache["nc"] = _build_program()
    nc = _cache["nc"]

    in_maps = _make_in_maps(p, q)

    from concourse.bass_utils import run_bass_kernel_spmd

    res = run_bass_kernel_spmd(nc, in_maps, list(range(N_CORES)))
    total = 0.0
    for r_ in res.results:
        total += r_["partial"].astype(np.float64).sum()
    return np.float32(total / B)


# revision 4
# speedup vs baseline: 1.6756x; 1.4832x over previous
"""TRN2 Bass/Tile kernel for nn_Loss_58317065945194.

Loss: per-sample EMD with r=2 over C=10 channels:
    d = p - q                       # [B, C]
    S = cumsum(d, axis=1)           # per-sample prefix sums
    per_sample = sqrt(mean(S**2))   # [B]
    out = mean(per_sample)          # scalar

Strategy (pure data parallel, 8 cores):
  - Shard B across 8 cores. Host casts to fp16 and lays each core's
    shard out channel-major: partition row = [C=10 planes][W=2048
    samples], so every on-device op is a 2D unit-stride fp16 access
    pattern (tensor_tensor runs in 2x_1p mode).
  - Per channel plane c (pipelined at plane granularity):
      * DMA p_c, q_c ([128, 2048] fp16 each)
      * Vector/GpSimd: d_c = p_c - q_c  (tensor_tensor)
      * Vector: S_c = d_c + S_{c-1} in place (chained prefix adds --
        no scan, no boundary fixup)
      * ACT: sq_c = S_c^2
      * Vector: U += sq_c (fp16 accumulator)
  - ACT: loss = sqrt(U / C), accum_out -> batch partial [128, 1].
  - Host sums the 8 cores' partials and divides by B.
"""

import sys

import numpy as np

if "/opt/trn_rl_repo" not in sys.path:
    sys.path.insert(0, "/opt/trn_rl_repo")

N_CORES = 8
B, C = 2097152, 10
BS = B // N_CORES        # samples per core shard (262144)
P = 128                  # SBUF partitions
W = BS // P              # samples per partition = plane width (2048)
FPP = W * C              # elems per partition (20480)
G_SUB_PLANES = ()        # planes whose subtract runs on GpSimd

_cache = {}


def _build_program():
    import concourse.tile as tile
    from concourse import bacc, mybir

    f32, f16 = mybir.dt.float32, mybir.dt.float16
    Alu = mybir.AluOpType
    Act = mybir.ActivationFunctionType

    nc = bacc.Bacc(
        "TRN2", target_bir_lowering=False, debug=False, num_devices=N_CORES
    )
    p_d = nc.dram_tensor("p", [P, FPP], f16, kind="ExternalInput").ap()
    q_d = nc.dram_tensor("q", [P, FPP], f16, kind="ExternalInput").ap()
    o_d = nc.dram_tensor("partial", [P, 1], f32, kind="ExternalOutput").ap()

    with tile.TileContext(nc) as tc:
        with (
            tc.tile_pool(name="io", bufs=1) as io,
            tc.tile_pool(name="work", bufs=1) as work,
            tc.tile_pool(name="small", bufs=1) as small,
        ):
            pt = [io.tile([P, W], f16, tag=f"p{c}", name=f"p{c}") for c in range(C)]
            qt = [io.tile([P, W], f16, tag=f"q{c}", name=f"q{c}") for c in range(C)]
            S = [work.tile([P, W], f16, tag=f"S{c}", name=f"S{c}") for c in range(C)]
            SQ = [work.tile([P, W], f16, tag=f"sq{c}", name=f"sq{c}") for c in range(C)]
            U = small.tile([P, W], f16, tag="U")
            acc = small.tile([P, 1], f32, tag="acc")

            for c in range(C):
                nc.sync.dma_start(pt[c][:], p_d[:, c * W : (c + 1) * W])
                nc.sync.dma_start(qt[c][:], q_d[:, c * W : (c + 1) * W])

            for c in range(C):
                # d_c = p_c - q_c
                eng = nc.gpsimd if c in G_SUB_PLANES else nc.vector
                eng.tensor_tensor(S[c][:], pt[c][:], qt[c][:], Alu.subtract)
                # S_c += S_{c-1}
                if c > 0:
                    nc.vector.tensor_tensor(
                        S[c][:], S[c][:], S[c - 1][:], Alu.add
                    )
                # sq_c = S_c^2
                nc.scalar.activation(SQ[c][:], S[c][:], Act.Square)
                # U accumulation
                if c == 1:
                    nc.vector.tensor_tensor(U[:], SQ[0][:], SQ[1][:], Alu.add)
                elif c > 1:
                    nc.vector.tensor_tensor(U[:], U[:], SQ[c][:], Alu.add)

            # loss[g] = sqrt(U[g] / C); acc = sum_g loss[g]
            lt = small.tile([P, W], f32, tag="loss")
            nc.scalar.activation(
                lt[:], U[:], Act.Sqrt, scale=1.0 / C, accum_out=acc[:]
            )
            nc.sync.dma_start(o_d[:], acc[:])
    nc.compile()
    return nc


def _make_in_maps(p, q):
    p = np.asarray(p, dtype=np.float32).reshape(B, C).astype(np.float16)
    q = np.asarray(q, dtype=np.float32).reshape(B, C).astype(np.float16)

    def prep(a, i):
        sh = a[i * BS : (i + 1) * BS].reshape(P, W, C)
        return np.ascontiguousarray(sh.transpose(0, 2, 1)).reshape(P, FPP)

    in_maps = []
    for i in range(N_CORES):
        in_maps.append({"p": prep(p, i), "q": prep(q, i)})
    return in_maps


def kernel(p, q, r):
    assert int(r) == 2, f"kernel specialized for r=2, got {r}"
    if "nc" not in _cache:
        _cache["nc"] = _build_program()
    nc = _cache["nc"]

    in_maps = _make_in_maps(p, q)

    from concourse.bass_utils import run_bass_kernel_spmd

    res = run_bass_kernel_spmd(nc, in_maps, list(range(N_CORES)))
    total = 0.0
    for r_ in res.results:
        total += r_["partial"].astype(np.float64).sum()
    return np.float32(total / B)
